# revision 9
# baseline (speedup 1.0000x reference)
"""Trainium2 Bass kernel for nn_EnhancedStrategySuperposition.

Grouped 16-expert MLP (Linear-SELU-LayerNorm-Linear-SELU-Linear-Tanh) with
softmax gating, B=32768, D=512, H1=128, H2=64, A=64.

Strategy: data-parallel over batch across 8 NeuronCores (4096 rows each),
weights replicated.  All compute in a transposed orientation (features on
partitions, batch rows in the free dimension) so no on-device transposes are
needed: the host pre-transposes state once.

Math folds (host-side):
  selu(u) = lam*max(u,0) + lam*alpha*(e^min(u,0)-1)
  We compute h* = max(u,0) + min(alpha*e^u, alpha)  (1 ACT exp + 2 DVE ops),
  so true h = lam*h* - lam*alpha.  The affine offset cancels inside the
  (shift-invariant) LayerNorm; the lam scale folds into eps and the next
  layer's weights:
    z2 = rstd* . (h* @ W2g - mu* x c) + b2f
  with W2g = gamma*W2, c = colsum(W2g), b2f = b2 + beta@W2,
  rstd* = 1/sqrt(var(h*) + eps/lam^2), mu* = mean(h*).
  Same trick for layer 2->3 with W3' = lam*W3, b3' = b3 - lam*alpha*colsum(W3).

Cross-partition moves (LN stats, broadcasts of per-row scalars) are done on
the PE with small constant matrices, because SBUF access patterns may only
start at partitions {0,32,64,96}.  Experts are processed in pairs packed
into the 128-partition dimension for layers 2/3 (each expert spans 64
partitions; the second matmul of a pair targets PSUM base 64).
"""

import os
import math
import numpy as np

import concourse.bass as bass
import concourse.tile as tile
import concourse.mybir as mybir
from concourse import bacc
from concourse._compat import with_exitstack
from concourse.bass_utils import run_bass_kernel_spmd

F32 = mybir.dt.float32
AF = mybir.ActivationFunctionType
OP = mybir.AluOpType

B_TOTAL = 32768
D = 512
K = 16
H1 = 128
H2 = 64
A = 64
NCORES = 8
BS = B_TOTAL // NCORES          # 4096 rows per core
TILE = 512                      # batch rows per inner tile
NT = BS // TILE                 # 8 tiles per core
NPAIR = K // 2

LAM = 1.0507009873554805
ALPHA = 1.6732632423543772
EPS = 1e-5
EPS2 = EPS / (LAM * LAM)

_CACHE = {}

LAST_RESULTS = None             # test.py reads exec_time_ns off this


# --------------------------------------------------------------------------
# device program
# --------------------------------------------------------------------------

@with_exitstack
def _kern(ctx, tc, io, y):
    nc = tc.nc

    cp = ctx.enter_context(tc.tile_pool(name="consts", bufs=1))
    xp = ctx.enter_context(tc.tile_pool(name="x", bufs=2))
    hp = ctx.enter_context(tc.tile_pool(name="h", bufs=K + 2))
    wa = ctx.enter_context(tc.tile_pool(name="wa", bufs=2))
    wb = ctx.enter_context(tc.tile_pool(name="wb", bufs=2))
    sp = ctx.enter_context(tc.tile_pool(name="st", bufs=2))
    op_ = ctx.enter_context(tc.tile_pool(name="out", bufs=2))
    ppa = ctx.enter_context(tc.tile_pool(name="psa", bufs=2, space="PSUM"))
    ppc = ctx.enter_context(tc.tile_pool(name="psc", bufs=1, space="PSUM"))
    ppb = ctx.enter_context(tc.tile_pool(name="psb", bufs=1, space="PSUM"))

    def cload(name, shape):
        t = cp.tile(shape, F32, tag=name)
        nc.sync.dma_start(t[:], io[name][:])
        return t

    w1 = cload("w1", [128, K * 4 * 128])
    w2 = cload("w2", [128, K * H2])
    w3 = cload("w3", [128, NPAIR * 128])
    foldc = cload("foldc", [128, A])
    gw = cload("gw", [128, 4 * K])
    zm = cload("zm", [128, K * K])
    la = cload("la", [K, NPAIR * 128])
    lbc = cload("lbc", [K, NPAIR * 128])
    b1e = cload("b1e", [128, K])
    b1a = cload("b1a", [128, K])
    b2e = cload("b2e", [128, NPAIR])
    b2a = cload("b2a", [128, NPAIR])
    b3e = cload("b3e", [128, NPAIR])
    gb = cload("gb", [K, 1])
    ones16 = cload("ones", [K, 1])
    e2c = cload("e2c", [K, 1])

    for t in range(NT):
        xs = xp.tile([128, 4, TILE], F32, tag="xt")
        nc.sync.dma_start(xs[:], io["xt"][:, :, t * TILE:(t + 1) * TILE])

        # ---- gate logits -> normalized gates gnorm [16, TILE] --------------
        gps = ppb.tile([K, TILE], F32, tag="gate")
        for c in range(4):
            nc.tensor.matmul(gps[:], gw[:, c * K:(c + 1) * K], xs[:, c, :],
                             start=(c == 0), stop=(c == 3))
        expg = sp.tile([K, TILE], F32, tag="expg")
        nc.scalar.activation(expg[:], gps[:], AF.Exp, bias=gb[:, 0:1])
        # denominator: reuse partition 0 of the gate psum bank
        nc.tensor.matmul(gps[0:1, :], ones16[:, 0:1], expg[:],
                         start=True, stop=True, skip_group_check=True)
        rec1 = sp.tile([1, TILE], F32, tag="rec1")
        nc.vector.reciprocal(rec1[:], gps[0:1, :])
        recb = sp.tile([K, TILE], F32, tag="recb")
        nc.gpsimd.partition_broadcast(recb[:], rec1[:])
        gnorm = sp.tile([K, TILE], F32, tag="gnorm")
        nc.vector.tensor_tensor(gnorm[:], expg[:], recb[:], OP.mult)

        # ---- pass A: expert layer-1 + LN stat matmuls ----------------------
        smu = ppb.tile([K, TILE], F32, tag="smu")
        ssq = ppb.tile([K, TILE], F32, tag="ssq")
        hts = []
        for k in range(K):
            ph = ppa.tile([128, TILE], F32, tag="ph")
            for c in range(4):
                nc.tensor.matmul(ph[:], w1[:, (k * 4 + c) * 128:(k * 4 + c + 1) * 128],
                                 xs[:, c, :], start=(c == 0), stop=(c == 3))
            e1 = wa.tile([128, TILE], F32, tag="e1")
            nc.scalar.activation(e1[:], ph[:], AF.Exp, bias=b1e[:, k:k + 1])
            r1 = wa.tile([128, TILE], F32, tag="r1")
            nc.vector.tensor_scalar(r1[:], ph[:], b1a[:, k:k + 1], 0.0, OP.add, OP.max)
            ht = hp.tile([128, TILE], F32, tag="ht")
            nc.vector.scalar_tensor_tensor(ht[:], e1[:], ALPHA, r1[:], OP.min, OP.add)
            hts.append(ht)
            hsq = wa.tile([128, TILE], F32, tag="hsq")
            nc.gpsimd.tensor_tensor(hsq[:], ht[:], ht[:], OP.mult)
            nc.tensor.matmul(smu[:], zm[:, k * K:(k + 1) * K], ht[:],
                             start=(k == 0), stop=(k == K - 1))
            nc.tensor.matmul(ssq[:], zm[:, k * K:(k + 1) * K], hsq[:],
                             start=(k == 0), stop=(k == K - 1))

        # ---- batched LN stats: mu* and rstd* [16, TILE] --------------------
        mu = sp.tile([K, TILE], F32, tag="mu")
        nc.scalar.activation(mu[:], smu[:], AF.Copy)
        sq = sp.tile([K, TILE], F32, tag="sq")
        nc.scalar.activation(sq[:], ssq[:], AF.Copy)
        m2 = sp.tile([K, TILE], F32, tag="m2")
        nc.vector.tensor_tensor(m2[:], mu[:], mu[:], OP.mult)
        veps = sp.tile([K, TILE], F32, tag="veps")
        nc.vector.scalar_tensor_tensor(veps[:], m2[:], -1.0, sq[:], OP.mult, OP.add)
        sg = sp.tile([K, TILE], F32, tag="sg")
        nc.scalar.activation(sg[:], veps[:], AF.Sqrt, bias=e2c[:, 0:1])
        rstd = sp.tile([K, TILE], F32, tag="rstd")
        nc.vector.reciprocal(rstd[:], sg[:])

        # ---- pass B: experts in pairs packed on 128 partitions -------------
        acc = op_.tile([128, TILE], F32, tag="acc")
        for j in range(NPAIR):
            ka, kb = 2 * j, 2 * j + 1
            # q = [h*_a @ W2g_a ; h*_b @ W2g_b] - mu* x c  (c folded into lbc)
            q = ppc.tile([128, TILE], F32, tag="q")
            nc.tensor.matmul(q[0:H2, :], w2[:, ka * H2:(ka + 1) * H2], hts[ka][:],
                             start=True, stop=False, skip_group_check=True)
            nc.tensor.matmul(q[H2:128, :], w2[:, kb * H2:(kb + 1) * H2], hts[kb][:],
                             start=True, stop=False, skip_group_check=True,
                             tile_position=(0, 64))
            nc.tensor.matmul(q[:], lbc[:, j * 128:(j + 1) * 128], mu[:],
                             start=False, stop=True, skip_group_check=True)
            # ab = broadcast of rstd*: rows 0-63 <- rstd[ka], 64-127 <- rstd[kb]
            abp = ppc.tile([128, TILE], F32, tag="ab")
            nc.tensor.matmul(abp[:], la[:, j * 128:(j + 1) * 128], rstd[:],
                             start=True, stop=True)
            absb = wb.tile([128, TILE], F32, tag="absb")
            nc.scalar.activation(absb[:], abp[:], AF.Copy)
            z2 = wb.tile([128, TILE], F32, tag="z2")
            nc.vector.tensor_tensor(z2[:], q[:], absb[:], OP.mult)
            e2 = wb.tile([128, TILE], F32, tag="e2")
            nc.scalar.activation(e2[:], z2[:], AF.Exp, bias=b2e[:, j:j + 1])
            r2 = wb.tile([128, TILE], F32, tag="r2")
            nc.vector.tensor_scalar(r2[:], z2[:], b2a[:, j:j + 1], 0.0, OP.add, OP.max)
            h2 = wb.tile([128, TILE], F32, tag="h2")
            nc.vector.scalar_tensor_tensor(h2[:], e2[:], ALPHA, r2[:], OP.min, OP.add)
            # layer 3: one full-width matmul per pair (block-diagonal weights)
            ep = ppc.tile([128, TILE], F32, tag="q")
            nc.tensor.matmul(ep[:], w3[:, j * 128:(j + 1) * 128], h2[:],
                             start=True, stop=True)
            eo = wb.tile([128, TILE], F32, tag="eo")
            nc.scalar.activation(eo[:], ep[:], AF.Tanh, bias=b3e[:, j:j + 1])
            # gates broadcast for the pair, then weighted accumulation
            gg = ppc.tile([128, TILE], F32, tag="gg")
            nc.tensor.matmul(gg[:], la[:, j * 128:(j + 1) * 128], gnorm[:],
                             start=True, stop=True)
            if j == 0:
                nc.vector.tensor_tensor(acc[:], eo[:], gg[:], OP.mult)
            else:
                pk = wb.tile([128, TILE], F32, tag="pk")
                nc.vector.tensor_tensor(pk[:], eo[:], gg[:], OP.mult)
                nc.vector.tensor_tensor(acc[:], acc[:], pk[:], OP.add)

        # fold the two 64-partition halves of acc with a PE matmul, DMA from PSUM
        fold = ppc.tile([A, TILE], F32, tag="gg")
        nc.tensor.matmul(fold[:], foldc[:, 0:A], acc[:], start=True, stop=True)
        yt = op_.tile([A, TILE], F32, tag="yt")
        nc.scalar.activation(yt[:], fold[:], AF.Copy)
        nc.sync.dma_start(y[:, t * TILE:(t + 1) * TILE], yt[:])


def _build():
    if "nc" in _CACHE:
        return _CACHE["nc"]
    nc = bacc.Bacc("TRN2", target_bir_lowering=False, debug=False,
                   enable_asserts=True, num_devices=NCORES)

    def din(name, shape):
        return nc.dram_tensor(name, shape, F32, kind="ExternalInput").ap()

    io = {
        "xt": din("xt", [128, 4, BS]),
        "w1": din("w1", [128, K * 4 * 128]),
        "w2": din("w2", [128, K * H2]),
        "w3": din("w3", [128, NPAIR * 128]),
        "foldc": din("foldc", [128, A]),
        "gw": din("gw", [128, 4 * K]),
        "zm": din("zm", [128, K * K]),
        "la": din("la", [K, NPAIR * 128]),
        "lbc": din("lbc", [K, NPAIR * 128]),
        "b1e": din("b1e", [128, K]),
        "b1a": din("b1a", [128, K]),
        "b2e": din("b2e", [128, NPAIR]),
        "b2a": din("b2a", [128, NPAIR]),
        "b3e": din("b3e", [128, NPAIR]),
        "gb": din("gb", [K, 1]),
        "ones": din("ones", [K, 1]),
        "e2c": din("e2c", [K, 1]),
    }
    y = nc.dram_tensor("y", [A, BS], F32, kind="ExternalOutput").ap()

    with tile.TileContext(nc) as tc:
        _kern(tc, io, y)
    nc.compile()
    _CACHE["nc"] = nc
    return nc


# --------------------------------------------------------------------------
# host-side preprocessing
# --------------------------------------------------------------------------

def _prep_consts(W1, b1, ln_gamma, ln_beta, W2, b2, W3, b3, gate_W, gate_b):
    f = np.float32
    W1 = np.asarray(W1, f)
    W2 = np.asarray(W2, f)
    W3 = np.asarray(W3, f)
    b1 = np.asarray(b1, f)
    b2 = np.asarray(b2, f)
    b3 = np.asarray(b3, f)
    ln_gamma = np.asarray(ln_gamma, f)
    ln_beta = np.asarray(ln_beta, f)
    gate_W = np.asarray(gate_W, f)
    gate_b = np.asarray(gate_b, f)

    lna = f(math.log(ALPHA))
    w1h = np.ascontiguousarray(
        W1.reshape(K, 4, 128, H1).transpose(2, 0, 1, 3).reshape(128, K * 4 * 128))
    W2g = ln_gamma[:, :, None] * W2                       # [K,H1,H2]
    w2h = np.ascontiguousarray(W2g.transpose(1, 0, 2).reshape(H1, K * H2))
    w3l = LAM * W3                                        # [K,H2,A]
    w3h = np.zeros((128, NPAIR * 128), f)
    for j in range(NPAIR):
        w3h[:H2, j * 128:j * 128 + A] = w3l[2 * j]
        w3h[H2:, j * 128 + A:(j + 1) * 128] = w3l[2 * j + 1]
    gwh = np.ascontiguousarray(
        gate_W.reshape(4, 128, K).transpose(1, 0, 2).reshape(128, 4 * K))

    zmh = np.zeros((128, K * K), f)
    for k in range(K):
        zmh[:, k * K + k] = 1.0 / H1

    cc = W2g.sum(axis=1)                                  # [K,H2]
    lah = np.zeros((K, NPAIR * 128), f)
    lbch = np.zeros((K, NPAIR * 128), f)
    for j in range(NPAIR):
        lah[2 * j, j * 128:j * 128 + H2] = 1.0
        lah[2 * j + 1, j * 128 + H2:(j + 1) * 128] = 1.0
        lbch[2 * j, j * 128:j * 128 + H2] = -cc[2 * j]
        lbch[2 * j + 1, j * 128 + H2:(j + 1) * 128] = -cc[2 * j + 1]

    b2f = b2 + np.einsum("kh,khm->km", ln_beta, W2)       # [K,H2]
    b3eff = b3 - (LAM * ALPHA) * W3.sum(axis=1)           # [K,A]

    def pair_pack(v):                                     # [K,64] -> [128,NPAIR]
        out = np.empty((128, NPAIR), f)
        for j in range(NPAIR):
            out[:H2, j] = v[2 * j]
            out[H2:, j] = v[2 * j + 1]
        return out

    return {
        "w1": w1h, "w2": w2h, "w3": w3h, "gw": gwh, "zm": zmh,
        "la": lah, "lbc": lbch,
        "b1e": np.ascontiguousarray((b1 + lna).T),
        "b1a": np.ascontiguousarray(b1.T),
        "b2e": pair_pack(b2f + lna),
        "b2a": pair_pack(b2f),
        "b3e": pair_pack(b3eff),
        "foldc": np.vstack([np.eye(A, dtype=f), np.eye(A, dtype=f)]),
        "gb": np.ascontiguousarray(gate_b.reshape(K, 1)),
        "ones": np.ones((K, 1), f),
        "e2c": np.full((K, 1), EPS2, f),
    }


def _prep_x(state_shard):
    # [BS, 512] -> [128, 4, BS]:  xt[p, c, b] = state[b, c*128 + p]
    a = np.asarray(state_shard, np.float32).T.reshape(4, 128, BS)
    return np.ascontiguousarray(a.transpose(1, 0, 2))


def kernel(state, W1, b1, ln_gamma, ln_beta, W2, b2, W3, b3, gate_W, gate_b):
    global LAST_RESULTS
    nc = _build()
    consts = _prep_consts(W1, b1, ln_gamma, ln_beta, W2, b2, W3, b3,
                          gate_W, gate_b)
    state = np.asarray(state, np.float32)
    in_maps = []
    for c in range(NCORES):
        m = dict(consts)
        m["xt"] = _prep_x(state[c * BS:(c + 1) * BS])
        in_maps.append(m)

    trace = os.environ.get("BASS_KERNEL_TRACE") == "1"
    res = run_bass_kernel_spmd(nc, in_maps, core_ids=list(range(NCORES)),
                               trace=trace)
    LAST_RESULTS = res
    out = np.empty((B_TOTAL, A), np.float32)
    for c in range(NCORES):
        out[c * BS:(c + 1) * BS] = res.results[c]["y"].T
    return out


if __name__ == "__main__":
    rng = np.random.default_rng(0)
    ins = {
        "state": rng.standard_normal((B_TOTAL, D)).astype(np.float32),
        "W1": (rng.standard_normal((K, D, H1)) / np.sqrt(D)).astype(np.float32),
        "b1": np.zeros((K, H1), np.float32),
        "ln_gamma": np.ones((K, H1), np.float32),
        "ln_beta": np.zeros((K, H1), np.float32),
        "W2": (rng.standard_normal((K, H1, H2)) / np.sqrt(H1)).astype(np.float32),
        "b2": np.zeros((K, H2), np.float32),
        "W3": (rng.standard_normal((K, H2, A)) / np.sqrt(H2)).astype(np.float32),
        "b3": np.zeros((K, A), np.float32),
        "gate_W": (rng.standard_normal((D, K)) / np.sqrt(D)).astype(np.float32),
        "gate_b": np.zeros((K,), np.float32),
    }
    y = kernel(**ins)
    print("ok", y.shape, y.dtype, float(np.abs(y).max()))


# revision 18
# speedup vs baseline: 75.3116x; 75.3116x over previous
"""Trainium2 Bass kernel for nn_EnhancedStrategySuperposition.

Grouped 16-expert MLP (Linear-SELU-LayerNorm-Linear-SELU-Linear-Tanh) with
softmax gating, B=32768, D=512, H1=128, H2=64, A=64.

Strategy: data-parallel over batch across 8 NeuronCores (4096 rows each),
weights replicated.  All compute in a transposed orientation (features on
partitions, batch rows in the free dimension); the host pre-transposes state
once, so no on-device transposes are needed.

Math folds (host-side):
  selu(u) = lam*max(u,0) + lam*alpha*(e^min(u,0)-1)
  We compute h* = max(u,0) + min(alpha*e^u, alpha)  (1 ACT exp + 2 DVE ops),
  so true h = lam*h* - lam*alpha.  The affine offset cancels inside the
  (shift-invariant) LayerNorm; the lam scale folds into eps and the next
  layer's weights:
    z2 = rstd* . (h* @ W2g - mu* x c) + b2f
  with W2g = gamma*W2, c = colsum(W2g), b2f = b2 + beta@W2,
  rstd* = 1/sqrt(var(h*) + eps/lam^2), mu* = mean(h*).
  Same trick for layer 2->3 with W3' = lam*W3, b3' = b3 - lam*alpha*colsum(W3).

Cross-partition moves (LN stats, broadcasts of per-row scalars) are done on
the PE with small constant matrices, because SBUF access patterns may only
start at partitions {0,32,64,96}.  Experts are processed in pairs packed
into the 128-partition dimension for layers 2/3.

Matmul precision is fp32 by default (BASS_KERNEL_MM=f32); float32r was
tried and crashes the exec unit on this runtime, bf16 trades ~0.7-1.4%
relative error for ~2x PE throughput (BASS_KERNEL_MM is the switch point).
"""

import os
import math
import numpy as np

import concourse.bass as bass
import concourse.tile as tile
import concourse.mybir as mybir
from concourse import bacc
from concourse._compat import with_exitstack
from concourse.bass_utils import run_bass_kernel_spmd

F32 = mybir.dt.float32
F32R = mybir.dt.float32r
AF = mybir.ActivationFunctionType
OP = mybir.AluOpType

B_TOTAL = 32768
D = 512
K = 16
H1 = 128
H2 = 64
A = 64
NCORES = 8
BS = B_TOTAL // NCORES          # 4096 rows per core
TILE = 512                      # batch rows per inner tile
NT = BS // TILE                 # 8 tiles per core
NPAIR = K // 2

LAM = 1.0507009873554805
ALPHA = 1.6732632423543772
EPS = 1e-5
EPS2 = EPS / (LAM * LAM)

# matmul input precision: "f32r" (fast, TF32-like) or "f32" (exact)
MM_MODE = os.environ.get("BASS_KERNEL_MM", "f32")

_CACHE = {}

LAST_RESULTS = None             # test.py reads exec_time_ns off this


# --------------------------------------------------------------------------
# device program
# --------------------------------------------------------------------------

@with_exitstack
def _kern(ctx, tc, io, y):
    nc = tc.nc

    MDT = F32R if MM_MODE == "f32r" else F32

    def mm(out, lhsT, rhs, **kw):
        nc.tensor.matmul(out, lhsT, rhs, **kw)

    cp = ctx.enter_context(tc.tile_pool(name="consts", bufs=1))
    xp = ctx.enter_context(tc.tile_pool(name="x", bufs=2))
    hp = ctx.enter_context(tc.tile_pool(name="h", bufs=K + 2))
    wa = ctx.enter_context(tc.tile_pool(name="wa", bufs=2))
    wb = ctx.enter_context(tc.tile_pool(name="wb", bufs=2))
    sp = ctx.enter_context(tc.tile_pool(name="st", bufs=2))
    op_ = ctx.enter_context(tc.tile_pool(name="out", bufs=2))
    ppa = ctx.enter_context(tc.tile_pool(name="psa", bufs=2, space="PSUM"))
    ppc = ctx.enter_context(tc.tile_pool(name="psc", bufs=1, space="PSUM"))
    ppb = ctx.enter_context(tc.tile_pool(name="psb", bufs=1, space="PSUM"))

    def cload(name, shape, dt=F32):
        t = cp.tile(shape, dt, tag=name)
        nc.sync.dma_start(t[:], io[name][:])
        return t

    w1 = cload("w1", [128, K * 4 * 128], MDT)
    w2 = cload("w2", [128, K * 128], MDT)
    w3 = cload("w3", [128, NPAIR * 128], MDT)
    foldc = cload("foldc", [128, A])
    gw = cload("gw", [128, 4 * K], MDT)
    zm = cload("zm", [128, K * K], MDT)
    la = cload("la", [K, NPAIR * 128], MDT)
    lbc = cload("lbc", [K, NPAIR * 128], MDT)
    b1e = cload("b1e", [128, K])
    b1a = cload("b1a", [128, K])
    b2e = cload("b2e", [128, NPAIR])
    b2a = cload("b2a", [128, NPAIR])
    b3e = cload("b3e", [128, NPAIR])
    gb = cload("gb", [K, 1])
    ones16 = cload("ones", [K, 1])
    e2c = cload("e2c", [K, 1])

    for t in range(NT * _CACHE.get("repeat", 1)):
        t = t % NT
        xs = xp.tile([128, 4, TILE], MDT, tag="xt")
        nc.sync.dma_start(xs[:], io["xt"][:, :, t * TILE:(t + 1) * TILE])

        # ---- gate logits -> normalized gates gnorm [16, TILE] --------------
        gps = ppb.tile([K, TILE], F32, tag="gate")
        for c in range(4):
            mm(gps[:], gw[:, c * K:(c + 1) * K], xs[:, c, :],
               start=(c == 0), stop=(c == 3))
        expg = sp.tile([K, TILE], F32, tag="expg")
        nc.scalar.activation(expg[:], gps[:], AF.Exp, bias=gb[:, 0:1])
        # denominator: reuse partition 0 of the gate psum bank
        mm(gps[0:1, :], ones16[:, 0:1], expg[:],
           start=True, stop=True, skip_group_check=True)
        rec1 = sp.tile([1, TILE], F32, tag="rec1")
        nc.vector.reciprocal(rec1[:], gps[0:1, :])
        recb = sp.tile([K, TILE], F32, tag="recb")
        nc.gpsimd.partition_broadcast(recb[:], rec1[:])
        gnorm = sp.tile([K, TILE], MDT, tag="gnorm")
        nc.vector.tensor_tensor(gnorm[:], expg[:], recb[:], OP.mult)

        # ---- pass A: expert layer-1 + LN stat matmuls ----------------------
        smu = ppb.tile([K, TILE], F32, tag="smu")
        ssq = ppb.tile([K, TILE], F32, tag="ssq")
        hts = []
        for k in range(K):
            ph = ppa.tile([128, TILE], F32, tag="ph")
            for c in range(4):
                mm(ph[:], w1[:, (k * 4 + c) * 128:(k * 4 + c + 1) * 128],
                   xs[:, c, :], start=(c == 0), stop=(c == 3))
            e1 = wa.tile([128, TILE], F32, tag="e1")
            nc.scalar.activation(e1[:], ph[:], AF.Exp, bias=b1e[:, k:k + 1])
            r1 = wa.tile([128, TILE], F32, tag="r1")
            nc.vector.tensor_scalar(r1[:], ph[:], b1a[:, k:k + 1], 0.0, OP.add, OP.max)
            ht = hp.tile([128, TILE], MDT, tag="ht")
            nc.vector.scalar_tensor_tensor(ht[:], e1[:], ALPHA, r1[:], OP.min, OP.add)
            hts.append(ht)
            hsq = wa.tile([128, TILE], MDT, tag="hsq")
            nc.gpsimd.tensor_tensor(hsq[:], ht[:].bitcast(F32), ht[:].bitcast(F32), OP.mult)
            mm(smu[:], zm[:, k * K:(k + 1) * K], ht[:],
               start=(k == 0), stop=(k == K - 1))
            mm(ssq[:], zm[:, k * K:(k + 1) * K], hsq[:],
               start=(k == 0), stop=(k == K - 1))

        # ---- batched LN stats: mu* and rstd* [16, TILE] --------------------
        mu = sp.tile([K, TILE], MDT, tag="mu")
        nc.scalar.activation(mu[:], smu[:], AF.Copy)
        sq = sp.tile([K, TILE], F32, tag="sq")
        nc.scalar.activation(sq[:], ssq[:], AF.Copy)
        m2 = sp.tile([K, TILE], F32, tag="m2")
        nc.vector.tensor_tensor(m2[:], mu[:].bitcast(F32), mu[:].bitcast(F32), OP.mult)
        veps = sp.tile([K, TILE], F32, tag="veps")
        nc.vector.scalar_tensor_tensor(veps[:], m2[:], -1.0, sq[:], OP.mult, OP.add)
        sg = sp.tile([K, TILE], F32, tag="sg")
        nc.scalar.activation(sg[:], veps[:], AF.Sqrt, bias=e2c[:, 0:1])
        rstd = sp.tile([K, TILE], MDT, tag="rstd")
        with nc.allow_low_precision(reason="rstd feeds a reduced-precision matmul"):
            nc.vector.reciprocal(rstd[:], sg[:])

        # ---- pass B: experts in pairs packed on 128 partitions -------------
        acc = op_.tile([128, TILE], F32, tag="acc")
        for j in range(NPAIR):
            ka, kb = 2 * j, 2 * j + 1
            # q = [h*_a @ W2g_a ; h*_b @ W2g_b] - mu* x c  (c folded into lbc)
            q = ppc.tile([128, TILE], F32, tag="q")
            mm(q[:], w2[:, ka * 128:ka * 128 + 128], hts[ka][:],
               start=True, stop=False, skip_group_check=True)
            mm(q[:], w2[:, kb * 128:kb * 128 + 128], hts[kb][:],
               start=False, stop=False, skip_group_check=True)
            mm(q[:], lbc[:, j * 128:(j + 1) * 128], mu[:],
               start=False, stop=True, skip_group_check=True)
            # ab = broadcast of rstd*: rows 0-63 <- rstd[ka], 64-127 <- rstd[kb]
            abp = ppc.tile([128, TILE], F32, tag="ab")
            mm(abp[:], la[:, j * 128:(j + 1) * 128], rstd[:],
               start=True, stop=True)
            absb = wb.tile([128, TILE], F32, tag="absb")
            nc.scalar.activation(absb[:], abp[:], AF.Copy)
            z2 = wb.tile([128, TILE], F32, tag="z2")
            nc.vector.tensor_tensor(z2[:], q[:], absb[:], OP.mult)
            e2 = wb.tile([128, TILE], F32, tag="e2")
            nc.scalar.activation(e2[:], z2[:], AF.Exp, bias=b2e[:, j:j + 1])
            r2 = wb.tile([128, TILE], F32, tag="r2")
            nc.vector.tensor_scalar(r2[:], z2[:], b2a[:, j:j + 1], 0.0, OP.add, OP.max)
            h2 = wb.tile([128, TILE], MDT, tag="h2")
            nc.vector.scalar_tensor_tensor(h2[:], e2[:], ALPHA, r2[:], OP.min, OP.add)
            # layer 3: one full-width matmul per pair (block-diagonal weights)
            ep = ppc.tile([128, TILE], F32, tag="q")
            mm(ep[:], w3[:, j * 128:(j + 1) * 128], h2[:], start=True, stop=True)
            eo = wb.tile([128, TILE], F32, tag="eo")
            nc.scalar.activation(eo[:], ep[:], AF.Tanh, bias=b3e[:, j:j + 1])
            # gates broadcast for the pair, then weighted accumulation
            gg = ppc.tile([128, TILE], F32, tag="gg")
            mm(gg[:], la[:, j * 128:(j + 1) * 128], gnorm[:], start=True, stop=True)
            if j == 0:
                nc.vector.tensor_tensor(acc[:], eo[:], gg[:], OP.mult)
            else:
                pk = wb.tile([128, TILE], F32, tag="pk")
                nc.vector.tensor_tensor(pk[:], eo[:], gg[:], OP.mult)
                nc.vector.tensor_tensor(acc[:], acc[:], pk[:], OP.add)

        # fold the two 64-partition halves of acc with a PE matmul
        fold = ppc.tile([A, TILE], F32, tag="gg")
        mm(fold[:], foldc[:, 0:A], acc[:], start=True, stop=True)
        yt = op_.tile([A, TILE], F32, tag="yt")
        nc.scalar.activation(yt[:], fold[:], AF.Copy)
        nc.sync.dma_start(y[:, t * TILE:(t + 1) * TILE], yt[:])


def _build(repeat=1):
    key = ("nc", repeat, MM_MODE)
    if key in _CACHE:
        return _CACHE[key]
    _CACHE["repeat"] = repeat
    nc = bacc.Bacc("TRN2", target_bir_lowering=False, debug=False,
                   enable_asserts=True, num_devices=NCORES)

    MDT = F32R if MM_MODE == "f32r" else F32

    def din(name, shape, dt=F32):
        return nc.dram_tensor(name, shape, dt, kind="ExternalInput").ap()

    io = {
        "xt": din("xt", [128, 4, BS], MDT),
        "w1": din("w1", [128, K * 4 * 128], MDT),
        "w2": din("w2", [128, K * 128], MDT),
        "w3": din("w3", [128, NPAIR * 128], MDT),
        "foldc": din("foldc", [128, A]),
        "gw": din("gw", [128, 4 * K], MDT),
        "zm": din("zm", [128, K * K], MDT),
        "la": din("la", [K, NPAIR * 128], MDT),
        "lbc": din("lbc", [K, NPAIR * 128], MDT),
        "b1e": din("b1e", [128, K]),
        "b1a": din("b1a", [128, K]),
        "b2e": din("b2e", [128, NPAIR]),
        "b2a": din("b2a", [128, NPAIR]),
        "b3e": din("b3e", [128, NPAIR]),
        "gb": din("gb", [K, 1]),
        "ones": din("ones", [K, 1]),
        "e2c": din("e2c", [K, 1]),
    }
    y = nc.dram_tensor("y", [A, BS], F32, kind="ExternalOutput").ap()

    with tile.TileContext(nc) as tc:
        _kern(tc, io, y)
    nc.compile()
    _CACHE[key] = nc
    return nc


# --------------------------------------------------------------------------
# host-side preprocessing
# --------------------------------------------------------------------------

def _prep_consts(W1, b1, ln_gamma, ln_beta, W2, b2, W3, b3, gate_W, gate_b):
    f = np.float32
    W1 = np.asarray(W1, f)
    W2 = np.asarray(W2, f)
    W3 = np.asarray(W3, f)
    b1 = np.asarray(b1, f)
    b2 = np.asarray(b2, f)
    b3 = np.asarray(b3, f)
    ln_gamma = np.asarray(ln_gamma, f)
    ln_beta = np.asarray(ln_beta, f)
    gate_W = np.asarray(gate_W, f)
    gate_b = np.asarray(gate_b, f)

    lna = f(math.log(ALPHA))
    w1h = np.ascontiguousarray(
        W1.reshape(K, 4, 128, H1).transpose(2, 0, 1, 3).reshape(128, K * 4 * 128))
    W2g = ln_gamma[:, :, None] * W2                       # [K,H1,H2]
    w2h = np.zeros((H1, K * 128), f)
    for k in range(K):
        off = 0 if k % 2 == 0 else H2
        w2h[:, k * 128 + off:k * 128 + off + H2] = W2g[k]
    w3l = LAM * W3                                        # [K,H2,A]
    w3h = np.zeros((128, NPAIR * 128), f)
    for j in range(NPAIR):
        w3h[:H2, j * 128:j * 128 + A] = w3l[2 * j]
        w3h[H2:, j * 128 + A:(j + 1) * 128] = w3l[2 * j + 1]
    gwh = np.ascontiguousarray(
        gate_W.reshape(4, 128, K).transpose(1, 0, 2).reshape(128, 4 * K))

    zmh = np.zeros((128, K * K), f)
    for k in range(K):
        zmh[:, k * K + k] = 1.0 / H1

    cc = W2g.sum(axis=1)                                  # [K,H2]
    lah = np.zeros((K, NPAIR * 128), f)
    lbch = np.zeros((K, NPAIR * 128), f)
    for j in range(NPAIR):
        lah[2 * j, j * 128:j * 128 + H2] = 1.0
        lah[2 * j + 1, j * 128 + H2:(j + 1) * 128] = 1.0
        lbch[2 * j, j * 128:j * 128 + H2] = -cc[2 * j]
        lbch[2 * j + 1, j * 128 + H2:(j + 1) * 128] = -cc[2 * j + 1]

    b2f = b2 + np.einsum("kh,khm->km", ln_beta, W2)       # [K,H2]
    b3eff = b3 - (LAM * ALPHA) * W3.sum(axis=1)           # [K,A]

    def pair_pack(v):                                     # [K,64] -> [128,NPAIR]
        out = np.empty((128, NPAIR), f)
        for j in range(NPAIR):
            out[:H2, j] = v[2 * j]
            out[H2:, j] = v[2 * j + 1]
        return out

    return {
        "w1": w1h, "w2": w2h, "w3": w3h, "gw": gwh, "zm": zmh,
        "la": lah, "lbc": lbch,
        "foldc": np.vstack([np.eye(A, dtype=f), np.eye(A, dtype=f)]),
        "b1e": np.ascontiguousarray((b1 + lna).T),
        "b1a": np.ascontiguousarray(b1.T),
        "b2e": pair_pack(b2f + lna),
        "b2a": pair_pack(b2f),
        "b3e": pair_pack(b3eff),
        "gb": np.ascontiguousarray(gate_b.reshape(K, 1)),
        "ones": np.ones((K, 1), f),
        "e2c": np.full((K, 1), EPS2, f),
    }


def _prep_x(state_shard):
    # [BS, 512] -> [128, 4, BS]:  xt[p, c, b] = state[b, c*128 + p]
    a = np.asarray(state_shard, np.float32).T.reshape(4, 128, BS)
    return np.ascontiguousarray(a.transpose(1, 0, 2))


def kernel(state, W1, b1, ln_gamma, ln_beta, W2, b2, W3, b3, gate_W, gate_b):
    global LAST_RESULTS
    nc = _build()
    consts = _prep_consts(W1, b1, ln_gamma, ln_beta, W2, b2, W3, b3,
                          gate_W, gate_b)
    state = np.asarray(state, np.float32)
    in_maps = []
    for c in range(NCORES):
        m = dict(consts)
        m["xt"] = _prep_x(state[c * BS:(c + 1) * BS])
        in_maps.append(m)

    trace = os.environ.get("BASS_KERNEL_TRACE") == "1"
    res = run_bass_kernel_spmd(nc, in_maps, core_ids=list(range(NCORES)),
                               trace=trace)
    LAST_RESULTS = res
    out = np.empty((B_TOTAL, A), np.float32)
    for c in range(NCORES):
        out[c * BS:(c + 1) * BS] = res.results[c]["y"].T
    return out


if __name__ == "__main__":
    rng = np.random.default_rng(0)
    ins = {
        "state": rng.standard_normal((B_TOTAL, D)).astype(np.float32),
        "W1": (rng.standard_normal((K, D, H1)) / np.sqrt(D)).astype(np.float32),
        "b1": np.zeros((K, H1), np.float32),
        "ln_gamma": np.ones((K, H1), np.float32),
        "ln_beta": np.zeros((K, H1), np.float32),
        "W2": (rng.standard_normal((K, H1, H2)) / np.sqrt(H1)).astype(np.float32),
        "b2": np.zeros((K, H2), np.float32),
        "W3": (rng.standard_normal((K, H2, A)) / np.sqrt(H2)).astype(np.float32),
        "b3": np.zeros((K, A), np.float32),
        "gate_W": (rng.standard_normal((D, K)) / np.sqrt(D)).astype(np.float32),
        "gate_b": np.zeros((K,), np.float32),
    }
    y = kernel(**ins)
    print("ok", y.shape, y.dtype, float(np.abs(y).max()))
